# revision 1
# baseline (speedup 1.0000x reference)
"""Trainium2 Bass kernel for a 5-layer gated graph conv (GatedGraphConv-style).

Math per layer (reference):
    m    = h @ W[l]                                   # [N, D]
    msgs = m[src] * edge_attr[:, None]                # [E, D]
    agg  = segment_sum(msgs, dst, N)                  # [N, D]
    h    = GRUCell(agg, h)                            # [N, D]

Distribution over 8 NeuronCores (node/dst-edge sharding):
    - Core c owns a padded slice of NPC=2560 nodes; it computes m for its
      slice, AllGathers the full bf16 m, gathers the sources of the edges
      whose dst lives in its slice (dma_gather from DRAM), and performs the
      edge-weighted scatter-add as a sequence of PE matmuls against host-built
      one-hot "selection" matrices S (S[e, d_local] = edge_attr[e]), which
      directly produce the aggregate in transposed layout aggT[d, node].
    - The GRU (two [*,256]x[256,768] matmuls + elementwise) runs on the local
      slice only, in transposed layout (gates on partitions, nodes on the free
      dim) so the per-gate biases become per-partition activation biases.

Host-side work is limited to layout transforms: edge sorting/padding, building
S / gather-index tables, transposing x and the weights, and unpacking the
output. All FLOPs of the reference computation run on device.
"""

import math
import os

import numpy as np
import ml_dtypes

import concourse.bass as bass
import concourse.tile as tile
from concourse import bacc, mybir
from concourse.bass_utils import run_bass_kernel_spmd

P = 128
CORES = 8
D = 256
KD = D // P          # 2 contraction chunks
GATES = 3 * D // P   # 6 gate chunks (r, z, n x 2)
NBW = 512            # node-block width (moving free dim)
G = 16               # chunks per gather / S group

AF = mybir.ActivationFunctionType
ALU = mybir.AluOpType

bf16 = ml_dtypes.bfloat16


class _Cfg:
    def __init__(self, n_nodes, n_layers, k_ch):
        self.N = n_nodes
        self.L = n_layers
        self.NPC = int(math.ceil(n_nodes / CORES / NBW)) * NBW  # padded nodes/core
        self.N_PAD = self.NPC * CORES
        self.NT = self.NPC // P    # dst tiles per core
        self.NB = self.NPC // NBW  # node blocks per core
        self.K_CH = k_ch           # chunks per dst tile (uniform)
        self.NCH = self.NT * k_ch
        self.NG = (self.NCH + G - 1) // G
        self.NCH_PAD = self.NG * G


def _preprocess(cfg, edge_index, edge_attr):
    """Sort/shard edges, build per-core S matrices and gather index tables."""
    src = np.asarray(edge_index[0], dtype=np.int64)
    dst = np.asarray(edge_index[1], dtype=np.int64)
    ea = np.asarray(edge_attr, dtype=np.float32)

    per_core = []
    k_ch = 1
    for c in range(CORES):
        base = c * cfg.NPC
        m = (dst >= base) & (dst < base + cfg.NPC)
        s_c, d_c, e_c = src[m], dst[m] - base, ea[m]
        order = np.argsort(d_c, kind="stable")
        s_c, d_c, e_c = s_c[order], d_c[order], e_c[order]
        tile_id = d_c // P
        counts = np.bincount(tile_id, minlength=cfg.NT).astype(np.int64)
        starts = np.concatenate([[0], np.cumsum(counts)])
        per_core.append((s_c, d_c, e_c, counts, starts))
        if counts.max(initial=0) > 0:
            k_ch = max(k_ch, int(np.max((counts + P - 1) // P)))
    cfg.K_CH = k_ch
    cfg.NCH = cfg.NT * k_ch
    cfg.NG = (cfg.NCH + G - 1) // G
    cfg.NCH_PAD = cfg.NG * G

    outs = []
    for c in range(CORES):
        s_c, d_c, e_c, counts, starts = per_core[c]
        src_pad = np.zeros((cfg.NT, cfg.K_CH * P), np.int64)
        ea_pad = np.zeros((cfg.NT, cfg.K_CH * P), np.float32)
        dloc_pad = np.zeros((cfg.NT, cfg.K_CH * P), np.int64)
        for t in range(cfg.NT):
            n = int(counts[t])
            sl = slice(int(starts[t]), int(starts[t]) + n)
            src_pad[t, :n] = s_c[sl]
            ea_pad[t, :n] = e_c[sl]
            dloc_pad[t, :n] = d_c[sl] - t * P

        S = np.zeros((cfg.NCH, P, P), np.float32)
        dl = dloc_pad.reshape(cfg.NCH, P)
        eaq = ea_pad.reshape(cfg.NCH, P)
        S[np.arange(cfg.NCH)[:, None], np.arange(P)[None, :], dl] = eaq
        if cfg.NCH_PAD > cfg.NCH:
            S = np.concatenate(
                [S, np.zeros((cfg.NCH_PAD - cfg.NCH, P, P), np.float32)], axis=0
            )
        S_g = (
            S.reshape(cfg.NG, G, P, P)
            .transpose(0, 2, 1, 3)
            .reshape(cfg.NG, P, G * P)
            .astype(bf16)
        )

        gsrc = np.zeros((cfg.NCH_PAD * P,), np.int64)
        gsrc[: cfg.NCH * P] = src_pad.reshape(-1)
        # dma_gather index packing: linear row i -> idxs[i % 16, i // 16],
        # replicated across the 8 GPSIMD cores (partition groups of 16).
        idx = np.zeros((P, cfg.NG * G * 8), np.int16)
        for g in range(cfg.NG):
            arr = gsrc[g * G * P : (g + 1) * G * P]
            pk = arr.reshape(G * 8, 16).T.astype(np.int16)  # [16, G*8]
            idx[:, g * G * 8 : (g + 1) * G * 8] = np.tile(pk, (8, 1))
        outs.append({"S": S_g, "idx": idx})
    return outs


def _build(cfg):
    phases = os.environ.get("GGC_PHASES", "ABCD")
    nc = bacc.Bacc("TRN2", target_bir_lowering=False, debug=False, num_devices=CORES)
    dt = mybir.dt
    NPC, NT, NB, L = cfg.NPC, cfg.NT, cfg.NB, cfg.L
    K_CH, NCH, NG = cfg.K_CH, cfg.NCH, cfg.NG

    xT_in = nc.dram_tensor("xT", [KD, P, NPC], dt.float32, kind="ExternalInput").ap()
    W_in = nc.dram_tensor("W", [L, KD, P, D], dt.float32, kind="ExternalInput").ap()
    wih_in = nc.dram_tensor("wihT", [KD, P, 3 * D], dt.float32, kind="ExternalInput").ap()
    whh_in = nc.dram_tensor("whhT", [KD, P, 3 * D], dt.float32, kind="ExternalInput").ap()
    b_in = nc.dram_tensor("bias", [P, 8], dt.float32, kind="ExternalInput").ap()
    S_in = nc.dram_tensor("S", [NG, P, G * P], dt.bfloat16, kind="ExternalInput").ap()
    idx_in = nc.dram_tensor("idx", [P, NG * G * 8], dt.int16, kind="ExternalInput").ap()
    out_hT = nc.dram_tensor("out_hT", [KD, P, NPC], dt.float32, kind="ExternalOutput").ap()

    # Collectives must execute in a deterministic serial order on all cores;
    # sharing one buffer serializes them via data deps (two buffers hung NRT).
    nbuf = 1 if os.environ.get("GGC_SINGLEBUF", "1") == "1" else 2
    m_c = [nc.dram_tensor(f"m_c{i}", [NPC, D], dt.bfloat16) for i in range(nbuf)]
    m_full = [
        nc.dram_tensor(f"m_full{i}", [cfg.N_PAD, D], dt.bfloat16, addr_space="Shared")
        for i in range(nbuf)
    ]
    rg = [list(range(CORES))]

    with tile.TileContext(nc) as tc:
        with (
            tc.tile_pool(name="const", bufs=1) as constp,
            tc.tile_pool(name="h", bufs=2) as hp,
            tc.tile_pool(name="agg", bufs=2) as aggp,
            tc.tile_pool(name="msg", bufs=3) as msgp,
            tc.tile_pool(name="sS", bufs=3) as sp,
            tc.tile_pool(name="mout", bufs=3) as moutp,
            tc.tile_pool(name="gtmp", bufs=2) as gtp,
            tc.tile_pool(name="pssc", bufs=2, space="PSUM") as psscp,
            tc.tile_pool(name="psg", bufs=2, space="PSUM") as psgp,
        ):
            # ---- constants ----
            W_sb = []
            for l in range(L):
                row = []
                for k in range(KD):
                    w = constp.tile([P, D], dt.float32, tag=f"W{l}_{k}", name=f"W{l}_{k}")
                    nc.sync.dma_start(w[:], W_in[l, k])
                    row.append(w)
                W_sb.append(row)
            wih_sb, whh_sb = [], []
            for k in range(KD):
                a = constp.tile([P, 3 * D], dt.float32, tag=f"wih{k}", name=f"wih{k}")
                nc.sync.dma_start(a[:], wih_in[k])
                wih_sb.append(a)
                b = constp.tile([P, 3 * D], dt.float32, tag=f"whh{k}", name=f"whh{k}")
                nc.sync.dma_start(b[:], whh_in[k])
                whh_sb.append(b)
            b_sb = constp.tile([P, 8], dt.float32, tag="bias", name="b_sb")
            nc.sync.dma_start(b_sb[:], b_in[:])
            idx_sb = constp.tile([P, NG * G * 8], dt.int16, tag="idx", name="idx_sb")
            nc.sync.dma_start(idx_sb[:], idx_in[:])

            # ---- initial h (transposed layout) ----
            h_t = [[None] * NB for _ in range(KD)]
            for k in range(KD):
                for nb in range(NB):
                    ht = hp.tile([P, NBW], dt.float32, tag=f"h{k}_{nb}", name=f"h{k}_{nb}")
                    nc.sync.dma_start(ht[:], xT_in[k, :, nb * NBW : (nb + 1) * NBW])
                    h_t[k][nb] = ht

            for l in range(L):
                mc, mf = m_c[l % nbuf], m_full[l % nbuf]

                # ---- A: m = h @ W[l]  (natural layout, bf16, to DRAM) ----
                for nt in range(NT):
                    ps = psgp.tile([P, NBW], dt.float32, tag="gia", name="ps_m")
                    nb, col = divmod(nt, NBW // P)
                    col *= P
                    for k in range(KD):
                        nc.tensor.matmul(
                            ps[:, :D],
                            lhsT=h_t[k][nb][:, col : col + P],
                            rhs=W_sb[l][k][:],
                            start=(k == 0),
                            stop=(k == KD - 1),
                        )
                    msb = moutp.tile([P, D], dt.bfloat16, tag="msb", name="msb")
                    nc.scalar.activation(msb[:], ps[:, :D], AF.Copy)
                    nc.sync.dma_start(mc[nt * P : (nt + 1) * P, :], msb[:])

                # ---- B: AllGather m ----
                if "B" in phases:
                    nc.gpsimd.collective_compute(
                        "AllGather", ALU.bypass, replica_groups=rg, ins=[mc[:]], outs=[mf[:]]
                    )
                if "C" not in phases:
                    if l == L - 1:
                        for k in range(KD):
                            for nb in range(NB):
                                nc.sync.dma_start(
                                    out_hT[k, :, nb * NBW : (nb + 1) * NBW], h_t[k][nb][:]
                                )
                    continue

                # ---- C: gather + scatter-matmul -> aggT ----
                agg_t = [
                    [
                        aggp.tile([P, NBW], dt.float32, tag=f"agg{k}_{nb}", name=f"agg{k}_{nb}")
                        for nb in range(NB)
                    ]
                    for k in range(KD)
                ]
                ps_sc = [None, None]
                for g in range(NG):
                    mt = msgp.tile([P, G, D], dt.bfloat16, tag="msg", name="mt")
                    if os.environ.get("GGC_NOGATHER", "0") == "1":
                        nc.sync.dma_start(
                            mt[:, 0, :], mf[(g % 2) * P : (g % 2) * P + P, :]
                        )
                    else:
                        nc.gpsimd.dma_gather(
                            out_ap=mt[:],
                            in_ap=mf[:],
                            idxs_ap=idx_sb[:, g * G * 8 : (g + 1) * G * 8],
                            num_idxs=G * P,
                            num_idxs_reg=G * P,
                            elem_size=D,
                            single_packet=False,
                        )
                    st = sp.tile([P, G * P], dt.bfloat16, tag="S", name="st")
                    nc.sync.dma_start(st[:], S_in[g])
                    for j in range(G):
                        q = g * G + j
                        if q >= NCH:
                            break
                        t, jj = divmod(q, K_CH)
                        if jj == 0:
                            for k in range(KD):
                                ps_sc[k] = psscp.tile(
                                    [P, P], dt.float32, tag=f"sc{k}", name=f"ps_sc{k}"
                                )
                        for k in range(KD):
                            nc.tensor.matmul(
                                ps_sc[k][:],
                                lhsT=mt[:, j, k * P : (k + 1) * P],
                                rhs=st[:, j * P : (j + 1) * P],
                                start=(jj == 0),
                                stop=(jj == K_CH - 1),
                            )
                        if jj == K_CH - 1:
                            nb, col = divmod(t, NBW // P)
                            col *= P
                            for k in range(KD):
                                nc.scalar.activation(
                                    agg_t[k][nb][:, col : col + P],
                                    ps_sc[k][:],
                                    AF.Copy,
                                )

                if "D" not in phases:
                    if l == L - 1:
                        for k in range(KD):
                            for nb in range(NB):
                                nc.sync.dma_start(
                                    out_hT[k, :, nb * NBW : (nb + 1) * NBW], agg_t[k][nb][:]
                                )
                    continue

                # ---- D: GRU on the local slice (transposed layout) ----
                for nb in range(NB):
                    rt, zt, ntl = {}, {}, {}
                    for gch in range(GATES):
                        pa = psgp.tile([P, NBW], dt.float32, tag="gia", name="pa")
                        for k in range(KD):
                            nc.tensor.matmul(
                                pa[:],
                                lhsT=wih_sb[k][:, gch * P : (gch + 1) * P],
                                rhs=agg_t[k][nb][:],
                                start=(k == 0),
                                stop=(k == KD - 1),
                            )
                        pb = psgp.tile([P, NBW], dt.float32, tag="ghb", name="pb")
                        for k in range(KD):
                            nc.tensor.matmul(
                                pb[:],
                                lhsT=whh_sb[k][:, gch * P : (gch + 1) * P],
                                rhs=h_t[k][nb][:],
                                start=(k == 0),
                                stop=(k == KD - 1),
                            )
                        if gch < 4:  # r / z gate chunks (combined b_ih+b_hh bias)
                            tag = f"rs{gch}" if gch < 2 else f"zs{gch - 2}"
                            s_ = gtp.tile([P, NBW], dt.float32, tag=tag, name="rzs")
                            nc.vector.tensor_scalar_add(s_[:], pb[:], b_sb[:, gch : gch + 1])
                            nc.vector.tensor_tensor(s_[:], pa[:], s_[:], op=ALU.add)
                            nc.scalar.activation(s_[:], s_[:], AF.Sigmoid)
                            if gch < 2:
                                rt[gch] = s_
                            else:
                                zt[gch - 2] = s_
                        else:  # n gate chunks: n = tanh(gi_n + b_ih_n + r*(gh_n + b_hh_n))
                            k2 = gch - 4
                            hn = gtp.tile([P, NBW], dt.float32, tag=f"hn{k2}", name="hn")
                            nc.vector.tensor_scalar_add(hn[:], pb[:], b_sb[:, 6 + k2 : 7 + k2])
                            nc.vector.tensor_tensor(hn[:], rt[k2][:], hn[:], op=ALU.mult)
                            nc.vector.tensor_tensor(hn[:], pa[:], hn[:], op=ALU.add)
                            nc.scalar.activation(
                                hn[:], hn[:], AF.Tanh, bias=b_sb[:, 4 + k2 : 5 + k2]
                            )
                            ntl[k2] = hn
                    # h' = n + z * (h - n)
                    for k in range(KD):
                        d_ = gtp.tile([P, NBW], dt.float32, tag=f"d{k}", name="d_")
                        nc.vector.tensor_tensor(d_[:], h_t[k][nb][:], ntl[k][:], op=ALU.subtract)
                        nc.vector.tensor_tensor(d_[:], zt[k][:], d_[:], op=ALU.mult)
                        hnew = hp.tile([P, NBW], dt.float32, tag=f"h{k}_{nb}", name="hnew")
                        nc.vector.tensor_tensor(hnew[:], ntl[k][:], d_[:], op=ALU.add)
                        h_t[k][nb] = hnew
                        if l == L - 1:
                            nc.sync.dma_start(
                                out_hT[k, :, nb * NBW : (nb + 1) * NBW], hnew[:]
                            )

    nc.compile()
    return nc


_BUILD_CACHE = {}


def _get_built(key, cfg):
    if key not in _BUILD_CACHE:
        _BUILD_CACHE[key] = _build(cfg)
    return _BUILD_CACHE[key]


def run(x, edge_index, edge_attr, weight, w_ih, w_hh, b_ih, b_hh, trace=False):
    n_nodes = x.shape[0]
    n_layers = weight.shape[0]
    assert x.shape[1] == D and w_ih.shape == (3 * D, D)

    cfg = _Cfg(n_nodes, n_layers, 1)
    pre = _preprocess(cfg, edge_index, edge_attr)

    # weights / layouts (host-side transforms only)
    x_pad = np.zeros((cfg.N_PAD, D), np.float32)
    x_pad[:n_nodes] = np.asarray(x, np.float32)
    W_host = np.ascontiguousarray(
        np.asarray(weight, np.float32).reshape(n_layers, KD, P, D)
    )
    wihT = np.ascontiguousarray(np.asarray(w_ih, np.float32).T.reshape(KD, P, 3 * D))
    whhT = np.ascontiguousarray(np.asarray(w_hh, np.float32).T.reshape(KD, P, 3 * D))
    b_ih = np.asarray(b_ih, np.float32)
    b_hh = np.asarray(b_hh, np.float32)
    bias = np.zeros((P, 8), np.float32)
    brz = (b_ih + b_hh)[: 2 * D].reshape(4, P)
    bias[:, 0:4] = brz.T
    bias[:, 4:6] = b_ih[2 * D :].reshape(2, P).T
    bias[:, 6:8] = b_hh[2 * D :].reshape(2, P).T

    nc = _get_built((n_nodes, n_layers, cfg.K_CH, cfg.NG), cfg)

    in_maps = []
    for c in range(CORES):
        xT_c = np.ascontiguousarray(
            x_pad[c * cfg.NPC : (c + 1) * cfg.NPC].T.reshape(KD, P, cfg.NPC)
        )
        in_maps.append(
            {
                "xT": xT_c,
                "W": W_host,
                "wihT": wihT,
                "whhT": whhT,
                "bias": bias,
                "S": pre[c]["S"],
                "idx": pre[c]["idx"],
            }
        )

    try:
        res = run_bass_kernel_spmd(nc, in_maps, list(range(CORES)), trace=trace)
    except ModuleNotFoundError:
        # no NTFF profile hook in this container; run without tracing
        res = run_bass_kernel_spmd(nc, in_maps, list(range(CORES)), trace=False)

    h = np.zeros((cfg.N_PAD, D), np.float32)
    for c in range(CORES):
        o = res.results[c]["out_hT"]  # [KD, P, NPC]
        h[c * cfg.NPC : (c + 1) * cfg.NPC] = o.reshape(D, cfg.NPC).T
    return h[:n_nodes], res


def kernel(**inputs):
    h, _ = run(**inputs)
    return h

